# revision 6
# baseline (speedup 1.0000x reference)
"""BinaryConv2d (sign-binarized 3x3 conv, stride 1, pad 1) on 8 Trainium2 cores.

Input  x      [32, 128, 56, 56] f32
       weight [256, 128, 3, 3]  f32  (binarized with sign() before the conv)
       b      [256]             f32
Output        [32, 256, 56, 56] f32

Sharding: data-parallel over the batch dim (4 images per core), weights
replicated to all cores.

Device kernel: 1D Winograd F(4,3) along W. The width dim is tiled into 14
tiles of 4 output cols; each tile needs 6 input cols transformed into 6
t-points (v = B^T d, done on HOST in fp16 and shipped instead of x). The
height dim stays direct: 3 kh taps accumulate in PSUM. Per strip of 14
output rows: 18 matmuls (6 t-points x 3 kh) of free size 196, C=128
contraction, into 6 PSUM slots (2 per bank). The inverse transform
o = A^T m (4 outputs from 6 t-points) runs on DVE + Pool with bias folded
in via scalar_tensor_tensor. PE work is 6/12 of direct conv (2x fewer
cycles than the fp16 shift-matmul formulation); measured rel err ~1.7e-3
(fp16 transforms, f32 PSUM).
"""

import functools

import numpy as np

P = 128          # partitions == input channels
H = W = 56       # spatial
O = 256          # output channels
NT = 6           # F(4,3) t-points
KH = 3           # kernel rows (direct accumulation)
NJ = 14          # width tiles (4 out cols each)
VROWS = H + 2    # 58 transformed input rows (pad included)
R = 14           # output rows per strip
NSTRIP = H // R  # 4
FREE = R * NJ    # 196 matmul free size
N_CORES = 8
N_PER_CORE = 4   # batch 32 / 8 cores

# F(4,3), interpolation points [0, 1, -1, 2, -2, inf]
BT = np.array(
    [
        [4, 0, -5, 0, 1, 0],
        [0, -4, -4, 1, 1, 0],
        [0, 4, -4, -1, 1, 0],
        [0, -2, -1, 2, 1, 0],
        [0, 2, -1, -2, 1, 0],
        [0, 4, 0, -5, 0, 1],
    ],
    np.float64,
)
G = np.array(
    [
        [1 / 4, 0, 0],
        [-1 / 6, -1 / 6, -1 / 6],
        [-1 / 6, 1 / 6, -1 / 6],
        [1 / 24, 1 / 12, 1 / 6],
        [1 / 24, -1 / 12, 1 / 6],
        [0, 0, 1],
    ],
    np.float64,
)
# A^T = [[1,1,1,1,1,0], [0,1,-1,2,-2,0], [0,1,1,4,4,0], [0,1,-1,8,-8,1]]
# implemented on-device as:
#   a = (m1 + bias) - m2ch, c = (m1 + bias) + m2, bb = m3 - m4, d = m3 + m4
#   o0 = (m0 + d) + c;  o1 = 2*bb + a;  o2 = 4*d + c;  o3 = (8*bb + a) + m5


@functools.lru_cache(maxsize=1)
def _build_nc():
    import concourse.mybir as mybir
    import concourse.tile as tile
    from concourse import bacc

    f16 = mybir.dt.float16
    f32 = mybir.dt.float32
    add = mybir.AluOpType.add
    sub = mybir.AluOpType.subtract
    mult = mybir.AluOpType.mult

    nc = bacc.Bacc()
    # xp: host-transformed input v[n, c, t, row, j]
    xp = nc.declare_dram_parameter(
        "xp", [N_PER_CORE, P, NT, VROWS, NJ], f16, isOutput=False
    )
    # wt: winograd weights u[c, t, kh, o]
    wt = nc.declare_dram_parameter("wt", [P, NT, KH, O], f16, isOutput=False)
    bias = nc.declare_dram_parameter("bias", [O], f32, isOutput=False)
    out = nc.declare_dram_parameter(
        "out", [N_PER_CORE, O, H, W], f32, isOutput=True
    )
    xp_ap = xp[:]
    wt_ap = wt[:]
    bias_ap = bias[:]
    out_ap = out[:]

    with tile.TileContext(nc) as tc:
        with (
            tc.tile_pool(name="wpool", bufs=1) as wpool,
            tc.tile_pool(name="xpool", bufs=3) as xpool,
            tc.tile_pool(name="spool", bufs=3) as spool,
            tc.tile_pool(name="opool", bufs=4) as opool,
            tc.tile_pool(name="psum", bufs=2, space="PSUM") as pp,
        ):
            # Weights/bias on the scalar (ACT) DMA queue so they don't
            # serialize behind the image loads on sync.
            u_sb = wpool.tile([P, NT, KH, O], f16)
            nc.scalar.dma_start(u_sb[:, 0:3], wt_ap[:, 0:3])
            nc.scalar.dma_start(u_sb[:, 3:6], wt_ap[:, 3:6])
            b_sb = wpool.tile([P, 2], f32)
            nc.scalar.dma_start(b_sb[:], bias_ap.rearrange("(g p) -> p g", p=P))

            # PE warmup: dummy matmuls with no data deps run during the
            # initial DMA wait and ramp the PE clock before the real stream.
            warm_sb = wpool.tile([P, 448], f16)
            nc.gpsimd.memset(warm_sb[:], 0.0)
            warm_ps = pp.tile([P, 3, 512], f32, tag="mt")
            N_WARM = 16
            for i in range(N_WARM):
                nc.tensor.matmul(
                    warm_ps[:, 0, 0:448],
                    warm_sb[:, 0:P],
                    warm_sb[:],
                    start=(i == 0),
                    stop=(i == N_WARM - 1),
                )

            for n in range(N_PER_CORE):
                v_sb = xpool.tile([P, NT, VROWS, NJ], f16, tag="vc")
                # split the 1.25MB image load so the first strips start early
                nc.sync.dma_start(v_sb[:, 0:2], xp_ap[n, :, 0:2])
                nc.sync.dma_start(v_sb[:, 2:4], xp_ap[n, :, 2:4])
                nc.sync.dma_start(v_sb[:, 4:6], xp_ap[n, :, 4:6])
                for oh in range(2):
                    osl = slice(oh * P, (oh + 1) * P)
                    bsc = b_sb[:, oh : oh + 1]
                    for s in range(NSTRIP):
                        r0 = R * s
                        mt = pp.tile([P, 3, 512], f32, tag="mt")
                        mv = [
                            mt[:, t // 2, (t % 2) * FREE : (t % 2 + 1) * FREE]
                            for t in range(NT)
                        ]
                        for t in range(NT):
                            for kh in range(KH):
                                nc.tensor.matmul(
                                    mv[t],
                                    u_sb[:, t, kh, osl],
                                    v_sb[:, t, r0 + kh : r0 + kh + R, :],
                                    start=(kh == 0),
                                    stop=(kh == KH - 1),
                                )
                        m = [x.rearrange("p (r j) -> p r j", r=R) for x in mv]
                        ot = opool.tile([P, R, W], f32)
                        oc = ot.rearrange("p r (j f) -> p r j f", f=4)
                        sc = spool.tile([P, 9, R, NJ], f32, tag="sc")
                        e_, h_, g_, k_, a_, c_, bb, d_, t0 = (
                            sc[:, i] for i in range(9)
                        )
                        # GPSIMD can't touch PSUM; DVE can read one PSUM
                        # operand per op. ACT evicts m1..m4, DVE reads m0/m5.
                        nc.scalar.add(e_, m[1], bsc)   # e = m1 + bias
                        nc.scalar.copy(g_, m[2])       # g = m2
                        nc.scalar.copy(h_, m[3])       # h = m3
                        nc.scalar.copy(k_, m[4])       # k = m4
                        # DVE does all combines (GPSIMD lacks PSUM access
                        # and TensorScalarPtr); ~1.2us/group < PE 1.8us
                        nc.vector.tensor_sub(a_, e_, g_)              # a = e-g
                        nc.vector.tensor_add(c_, e_, g_)              # c = e+g
                        nc.vector.tensor_sub(bb, h_, k_)              # bb = h-k
                        nc.vector.tensor_add(d_, h_, k_)              # d = h+k
                        nc.vector.scalar_tensor_tensor(
                            oc[:, :, :, 1], bb, 2.0, a_, mult, add
                        )                                             # o1
                        t3 = oc[:, :, :, 3]
                        nc.vector.scalar_tensor_tensor(t3, bb, 8.0, a_, mult, add)
                        nc.vector.tensor_add(t3, t3, m[5])            # o3
                        nc.vector.scalar_tensor_tensor(               # t0 = m0+d
                            t0, m[0], 0.0, d_, add, add
                        )
                        nc.vector.tensor_add(oc[:, :, :, 0], t0, c_)  # o0
                        nc.vector.scalar_tensor_tensor(
                            oc[:, :, :, 2], d_, 4.0, c_, mult, add
                        )                                             # o2
                        nc.sync.dma_start(out_ap[n, osl, r0 : r0 + R, :], ot[:])
    nc.finalize()
    return nc


def _prep(x, weight, b):
    x = np.asarray(x, dtype=np.float32)
    w = np.asarray(weight, dtype=np.float32)
    b = np.ascontiguousarray(np.asarray(b, dtype=np.float32))
    bw = np.sign(w.astype(np.float64))
    N = x.shape[0]

    # weights: u[c, t, kh, o] = sum_s G[t,s] * sign(w)[o,c,kh,s]
    ut = np.einsum("ts,ocks->ctko", G, bw)
    ut = np.ascontiguousarray(ut).astype(np.float16)

    # input: pad W to 58 cols, transform width tiles: v[n,c,t,row,j]
    xpad = np.zeros((N, P, VROWS, VROWS), np.float16)
    xpad[:, :, 1 : H + 1, 1 : W + 1] = x.astype(np.float16)
    # seg[n,c,row,j,s] = xpad[n,c,row,4j+s]
    sh = xpad.strides
    seg = np.lib.stride_tricks.as_strided(
        xpad,
        shape=(N, P, VROWS, NJ, 6),
        strides=(sh[0], sh[1], sh[2], 4 * sh[3], sh[3]),
    )
    vp = np.einsum("ts,ncrjs->nctrj", BT, seg.astype(np.float32))
    vp = vp.astype(np.float16)
    return vp, ut, b


def _run(in_maps, trace=False):
    from concourse.bass_utils import run_bass_kernel_spmd

    nc = _build_nc()
    return run_bass_kernel_spmd(
        nc, in_maps, core_ids=list(range(N_CORES)), trace=trace
    )


def kernel(x, weight, b):
    vp, ut, bias = _prep(x, weight, b)
    in_maps = [
        {
            "xp": np.ascontiguousarray(vp[c * N_PER_CORE : (c + 1) * N_PER_CORE]),
            "wt": ut,
            "bias": bias,
        }
        for c in range(N_CORES)
    ]
    res = _run(in_maps, trace=False)
    return np.concatenate([r["out"] for r in res.results], axis=0)


# revision 9
# speedup vs baseline: 1.2026x; 1.2026x over previous
"""BinaryConv2d (sign-binarized 3x3 conv, stride 1, pad 1) on 8 Trainium2 cores.

Input  x      [32, 128, 56, 56] f32
       weight [256, 128, 3, 3]  f32  (binarized with sign() before the conv)
       b      [256]             f32
Output        [32, 256, 56, 56] f32

Sharding: data-parallel over the batch dim (4 images per core), weights
replicated to all cores.

Device kernel: 1D Winograd F(4,3) along W. Width is tiled into 14 tiles of
4 output cols; the 6-point input transform v = B^T d runs on HOST (fp16)
and is shipped instead of x. Height stays direct: 3 kh taps accumulate in
PSUM, so PE work is 6/12 of the direct fp16 shift-matmul conv. Per strip
of 28 output rows: 18 matmuls (6 t-points x 3 kh) of free 392 into 6
bank-aligned PSUM slots (3 two-bank tiles), plus one diag(bias) matmul
into the t1 slot (t1 has A-coeff +1 in every output, so bias rides the
accumulation for free). ACT evicts the 6 slots to fp16 SBUF in 3 big ops;
DVE forms the F(4,3) inverse-transform combos in fp16 and writes 3 of the
4 output columns in f32; GPSIMD writes the 4th. Measured rel err ~2e-3.
"""

import functools

import numpy as np

P = 128          # partitions == input channels
H = W = 56       # spatial
O = 256          # output channels
NT = 6           # F(4,3) t-points
KH = 3           # kernel rows (direct accumulation)
NJ = 14          # width tiles (4 out cols each)
VROWS = H + 2    # 58 transformed input rows (pad included)
R = 28           # output rows per strip
NSTRIP = H // R  # 2
FREE = R * NJ    # 392 matmul free size
N_CORES = 8
N_PER_CORE = 4   # batch 32 / 8 cores

# F(4,3), interpolation points [0, 1, -1, 2, -2, inf]
BT = np.array(
    [
        [4, 0, -5, 0, 1, 0],
        [0, -4, -4, 1, 1, 0],
        [0, 4, -4, -1, 1, 0],
        [0, -2, -1, 2, 1, 0],
        [0, 2, -1, -2, 1, 0],
        [0, 4, 0, -5, 0, 1],
    ],
    np.float64,
)
G = np.array(
    [
        [1 / 4, 0, 0],
        [-1 / 6, -1 / 6, -1 / 6],
        [-1 / 6, 1 / 6, -1 / 6],
        [1 / 24, 1 / 12, 1 / 6],
        [1 / 24, -1 / 12, 1 / 6],
        [0, 0, 1],
    ],
    np.float64,
)
# A^T = [[1,1,1,1,1,0], [0,1,-1,2,-2,0], [0,1,1,4,4,0], [0,1,-1,8,-8,1]]
# with e=m1(+bias), g=m2, h=m3, k=m4:
#   Q = e-g, P = e+g, S = h-k, Rr = h+k
#   o0 = (m0+P)+Rr;  o1 = 2S+Q;  o2 = 4Rr+P;  o3 = (8S+Q)+m5


@functools.lru_cache(maxsize=1)
def _build_nc():
    import concourse.mybir as mybir
    import concourse.tile as tile
    from concourse import bacc

    f16 = mybir.dt.float16
    f32 = mybir.dt.float32
    add = mybir.AluOpType.add
    mult = mybir.AluOpType.mult

    nc = bacc.Bacc()
    # xp: host-transformed input v[n, c, t, row, j]
    xp = nc.declare_dram_parameter(
        "xp", [N_PER_CORE, P, NT, VROWS, NJ], f16, isOutput=False
    )
    # wt: winograd weights u[c, t, kh, o]
    wt = nc.declare_dram_parameter("wt", [P, NT, KH, O], f16, isOutput=False)
    # bias: diag(b) stationaries per o-half: bias[p, oh, o] = b[oh*128+o]*(p==o)
    bias = nc.declare_dram_parameter("bias", [P, 2, P], f16, isOutput=False)
    out = nc.declare_dram_parameter(
        "out", [N_PER_CORE, O, H, W], f32, isOutput=True
    )
    xp_ap = xp[:]
    wt_ap = wt[:]
    bias_ap = bias[:]
    out_ap = out[:]

    with tile.TileContext(nc) as tc:
        with (
            tc.tile_pool(name="wpool", bufs=1) as wpool,
            tc.tile_pool(name="xpool", bufs=3) as xpool,
            tc.tile_pool(name="spool", bufs=3) as spool,
            tc.tile_pool(name="opool", bufs=4) as opool,
            tc.tile_pool(name="psum", bufs=4, space="PSUM") as pp,
        ):
            # Weights/bias on the scalar (ACT) DMA queue so they don't
            # serialize behind the image loads on sync.
            u_sb = wpool.tile([P, NT, KH, O], f16)
            nc.scalar.dma_start(u_sb[:, 0:3], wt_ap[:, 0:3])
            nc.scalar.dma_start(u_sb[:, 3:6], wt_ap[:, 3:6])
            bd_sb = wpool.tile([P, 2, P], f16)
            nc.scalar.dma_start(bd_sb[:], bias_ap)
            ones_sb = wpool.tile([P, FREE], f16)
            nc.gpsimd.memset(ones_sb[:], 1.0)

            # PE warmup: dummy matmuls with no data deps run during the
            # initial DMA wait and ramp the PE clock before the real stream.
            warm_sb = wpool.tile([P, 448], f16)
            nc.gpsimd.memset(warm_sb[:], 0.0)
            warm_ps = pp.tile([P, 2, 512], f32, tag="mt")
            N_WARM = 16
            for i in range(N_WARM):
                nc.tensor.matmul(
                    warm_ps[:, 0, 0:448],
                    warm_sb[:, 0:P],
                    warm_sb[:],
                    start=(i == 0),
                    stop=(i == N_WARM - 1),
                )

            for n in range(N_PER_CORE):
                v_sb = xpool.tile([P, NT, VROWS, NJ], f16, tag="vc")
                # split the 1.25MB image load so the first strips start early
                nc.sync.dma_start(v_sb[:, 0:2], xp_ap[n, :, 0:2])
                nc.sync.dma_start(v_sb[:, 2:4], xp_ap[n, :, 2:4])
                nc.sync.dma_start(v_sb[:, 4:6], xp_ap[n, :, 4:6])
                for oh in range(2):
                    osl = slice(oh * P, (oh + 1) * P)
                    for s in range(NSTRIP):
                        r0 = R * s
                        # PSUM slots: tA=[m1,m3], tB=[m2,m4], tC=[m0,m5]
                        tA = pp.tile([P, 2, 512], f32, tag="mt")
                        tB = pp.tile([P, 2, 512], f32, tag="mt")
                        tC = pp.tile([P, 2, 512], f32, tag="mt")
                        slot = {
                            1: tA[:, 0, 0:FREE], 3: tA[:, 1, 0:FREE],
                            2: tB[:, 0, 0:FREE], 4: tB[:, 1, 0:FREE],
                            0: tC[:, 0, 0:FREE], 5: tC[:, 1, 0:FREE],
                        }

                        def mms(t, extra_first=False):
                            if extra_first:  # bias rides the t=1 slot
                                nc.tensor.matmul(
                                    slot[t], bd_sb[:, oh], ones_sb[:],
                                    start=True, stop=False,
                                )
                            for kh in range(KH):
                                nc.tensor.matmul(
                                    slot[t],
                                    u_sb[:, t, kh, osl],
                                    v_sb[:, t, r0 + kh : r0 + kh + R, :],
                                    start=(kh == 0 and not extra_first),
                                    stop=(kh == KH - 1),
                                )

                        mms(1, extra_first=True)
                        mms(3)
                        mms(2)
                        mms(4)
                        mms(0)
                        mms(5)

                        # fp16 scratch: [e h | g k | z0 z5 | Q S | Pp Rr | u0 t3]
                        sc = spool.tile([P, 12, R, NJ], f16, tag="sc")
                        scf = sc.rearrange("p s r j -> p s (r j)")
                        # ACT: evict the six PSUM slots to fp16
                        nc.scalar.copy(sc[:, 0:2], tA[:, :, 0:FREE].rearrange(
                            "p s (r j) -> p s r j", r=R))
                        nc.scalar.copy(sc[:, 2:4], tB[:, :, 0:FREE].rearrange(
                            "p s (r j) -> p s r j", r=R))
                        nc.scalar.copy(sc[:, 4:6], tC[:, :, 0:FREE].rearrange(
                            "p s (r j) -> p s r j", r=R))
                        eh = scf[:, 0:2]     # [e, h]
                        gk = scf[:, 2:4]     # [g, k]
                        QS = scf[:, 6:8]     # [Q, S]
                        PR = scf[:, 8:10]    # [Pp, Rr]
                        Q, S = scf[:, 6], scf[:, 7]
                        Pp, Rr = scf[:, 8], scf[:, 9]
                        z0, z5 = scf[:, 4], scf[:, 5]
                        u0, t3 = scf[:, 10], scf[:, 11]
                        nc.vector.tensor_sub(QS, eh, gk)
                        nc.vector.tensor_add(PR, eh, gk)

                        ot = opool.tile([P, R, W], f32)
                        oc = ot.rearrange("p r (j f) -> p (r j) f", f=4)
                        # DVE: o1, o2, t3, o3; GPSIMD: u0, o0
                        nc.gpsimd.tensor_add(u0, z0, Pp)
                        nc.vector.scalar_tensor_tensor(
                            oc[:, :, 1], S, 2.0, Q, mult, add
                        )
                        nc.vector.scalar_tensor_tensor(t3, S, 8.0, Q, mult, add)
                        nc.vector.scalar_tensor_tensor(
                            oc[:, :, 2], Rr, 4.0, Pp, mult, add
                        )
                        nc.vector.tensor_add(oc[:, :, 3], t3, z5)
                        nc.gpsimd.tensor_add(oc[:, :, 0], u0, Rr)
                        nc.sync.dma_start(out_ap[n, osl, r0 : r0 + R, :], ot[:])
    nc.finalize()
    return nc


def _prep(x, weight, b):
    x = np.asarray(x, dtype=np.float32)
    w = np.asarray(weight, dtype=np.float32)
    b = np.asarray(b, dtype=np.float32)
    bw = np.sign(w.astype(np.float64))
    N = x.shape[0]

    # weights: u[c, t, kh, o] = sum_s G[t,s] * sign(w)[o,c,kh,s]
    ut = np.einsum("ts,ocks->ctko", G, bw)
    ut = np.ascontiguousarray(ut).astype(np.float16)

    # bias diag stationaries: bd[p, oh, o] = b[oh*128+o] if p==o
    bd = np.zeros((P, 2, P), np.float16)
    for ohalf in range(2):
        np.fill_diagonal(bd[:, ohalf, :], b[ohalf * P : (ohalf + 1) * P])

    # input: pad W to 58 cols, transform width tiles: v[n,c,t,row,j]
    xpad = np.zeros((N, P, VROWS, VROWS), np.float16)
    xpad[:, :, 1 : H + 1, 1 : W + 1] = x.astype(np.float16)
    sh = xpad.strides
    seg = np.lib.stride_tricks.as_strided(
        xpad,
        shape=(N, P, VROWS, NJ, 6),
        strides=(sh[0], sh[1], sh[2], 4 * sh[3], sh[3]),
    )
    vp = np.einsum("ts,ncrjs->nctrj", BT, seg.astype(np.float32))
    vp = vp.astype(np.float16)
    return vp, ut, bd


def _run(in_maps, trace=False):
    from concourse.bass_utils import run_bass_kernel_spmd

    nc = _build_nc()
    return run_bass_kernel_spmd(
        nc, in_maps, core_ids=list(range(N_CORES)), trace=trace
    )


def kernel(x, weight, b):
    vp, ut, bd = _prep(x, weight, b)
    in_maps = [
        {
            "xp": np.ascontiguousarray(vp[c * N_PER_CORE : (c + 1) * N_PER_CORE]),
            "wt": ut,
            "bias": bd,
        }
        for c in range(N_CORES)
    ]
    res = _run(in_maps, trace=False)
    return np.concatenate([r["out"] for r in res.results], axis=0)
